# revision 2
# baseline (speedup 1.0000x reference)
"""Trainium2 Bass kernel for nn_KernelConv (per-pixel dynamic 5x5 conv), v6.

  out[b,n,y,x] = W[b,n,y,x] * sum_{i,j} core[b, n*25+i*5+j, y, x] * frames_pad[b, n, y+i-2, x+j-2]

Sharding: pure data parallel; 16 (b,n) slices split 2-per-core across 8 cores.

v6 — y-pair packed fp16:
  On this rig the DMA sustains ~300GB/s only for the planar geometry (2KB
  descriptors, strided source); packed/contiguous and 1KB-fp16 layouts cap
  near 65-85GB/s. So all inputs are fp16 values packed TWO ADJACENT Y-ROWS
  per float32 word, keeping the exact fast descriptor geometry at half the
  bytes (measured: core load 166us f32 -> 98us y-paired).

  One SBUF partition row p holds output rows {2p, 2p+1} interleaved as
  (x, parity) pairs; a strip covers 256 output rows, 2 strips per slice.
  - host packs: corey[m,q,p,x] = pack2(core[m,q,2p,x], core[m,q,2p+1,x]);
    frY[m,i,p,c] = pack2(fpad[m,2p+i,c], fpad[m,2p+1+i,c]) (the 5 kernel-row
    shifts are host-materialized since engine partition bases are
    quadrant-locked); wY likewise.
  - products (fp16, via bitcast views of the f32 tiles) on DVE (4 kernel
    rows) + Pool/gpsimd (1 row); the j-shift is a stride-2 overlapping AP.
  - 25-plane reduction on the otherwise-idle PE: identity matmuls
    accumulating into a 2-bank PSUM tile (one bank per 512-wide half).
  - evacuation deinterleaves parity while multiplying by W (DVE), and the
    output store runs on the Activation HWDGE queue (2KB descriptors).
"""

import numpy as np

import concourse.bacc as bacc
import concourse.bass as bass
import concourse.mybir as mybir
import concourse.tile as tile

F32 = mybir.dt.float32
F16 = mybir.dt.float16

B, N, H, Wd = 2, 8, 512, 512
K = 5
K2 = K * K
SLICES_PER_CORE = 2
HP = H + 4
WP = Wd + 4
N_CORES = 8
HY = H // 2                   # 256 y-pair rows per slice
XP = 2 * Wd                   # (x, parity) fp16 elements per pair-row
POOL_I = 0                    # kernel-row block computed on Pool; rest on DVE


def _build_program(reps=1):
    nc = bacc.Bacc("TRN2", target_bir_lowering=False)
    core_d = nc.dram_tensor("corey", (SLICES_PER_CORE, K2, HY, Wd), F32, kind="ExternalInput")
    fr_d = nc.dram_tensor("fry", (SLICES_PER_CORE, K, HY, WP), F32, kind="ExternalInput")
    w_d = nc.dram_tensor("wy", (SLICES_PER_CORE, HY, Wd), F32, kind="ExternalInput")
    id_d = nc.dram_tensor("ident", (128, 128), F16, kind="ExternalInput")
    out_d = nc.dram_tensor("out", (SLICES_PER_CORE, H, Wd), F32, kind="ExternalOutput")

    with tile.TileContext(nc) as tc:
        with tc.tile_pool(name="const", bufs=1) as cpool, \
             tc.tile_pool(name="sbuf", bufs=3) as pool, \
             tc.psum_pool(name="ps", bufs=2) as pp:
            ident = cpool.tile([128, 128], F16, tag="ident")
            nc.sync.dma_start(out=ident, in_=id_d[:, :])

            for rep in range(reps):
                for s in range(SLICES_PER_CORE):
                    for p0 in (0, 128):
                        fw32 = pool.tile([128, K * WP], F32, tag="FW")
                        wt32 = pool.tile([128, Wd], F32, tag="Wt")
                        acc = pool.tile([128, XP], F32, tag="acc")
                        ps = pp.tile([128, XP], F32, tag="psum")

                        fwp = fw32.ap[0][0]
                        wtp = wt32.ap[0][0]
                        psp = ps.ap[0][0]
                        ap_ = acc.ap[0][0]
                        fw16 = fw32.tensor.bitcast(F16)
                        wt16 = wt32.tensor.bitcast(F16)

                        # frame windows (all 5 shifts): FW[p, i*WP+c] word
                        nc.sync.dma_start(
                            out=bass.AP(fw32.tensor, fw32.offset,
                                        [(fwp, 128), (WP, K), (1, WP)]),
                            in_=fr_d[s, :, p0:p0 + 128, :].transpose([1, 0, 2]))
                        nc.sync.dma_start(
                            out=bass.AP(wt32.tensor, wt32.offset, [(wtp, 128), (1, Wd)]),
                            in_=w_d[s, p0:p0 + 128, :])

                        order = [i for i in range(K) if i != POOL_I] + [POOL_I]
                        blocks = []
                        for i in order:
                            ct32 = pool.tile([128, K * Wd], F32, tag="C")
                            prod = pool.tile([128, K * XP], F16, tag="prod")
                            cp = ct32.ap[0][0]
                            ppr = prod.ap[0][0]
                            ct16 = ct32.tensor.bitcast(F16)
                            # core i-block: C[p, j*512+x] word = corey[s,5i+j,p0+p,x]
                            nc.sync.dma_start(
                                out=bass.AP(ct32.tensor, ct32.offset,
                                            [(cp, 128), (Wd, K), (1, Wd)]),
                                in_=core_d[s, i * K:(i + 1) * K,
                                           p0:p0 + 128, :].transpose([1, 0, 2]))
                            # product: prod[p, j*XP + 2x+par] =
                            #   C[p, (j*512+x)*2+par] * FW[p, (i*WP + x+j)*2+par]
                            eng = nc.gpsimd if i == POOL_I else nc.vector
                            eng.tensor_mul(
                                out=bass.AP(prod.tensor, prod.offset,
                                            [(ppr, 128), (XP, K), (1, XP)]),
                                in0=bass.AP(ct16, 2 * ct32.offset,
                                            [(2 * cp, 128), (XP, K), (1, XP)]),
                                in1=bass.AP(fw16, 2 * (fw32.offset + i * WP),
                                            [(2 * fwp, 128), (2, K), (1, XP)]))
                            blocks.append((i, prod, ppr))

                        # PE: accumulate the 25 product planes into 2 PSUM
                        # bank halves (512 f32 per partition each)
                        nmm = 0
                        for bi, (i, prod, ppr) in enumerate(blocks):
                            for j in range(K):
                                for half in range(2):
                                    nc.tensor.matmul(
                                        out=bass.AP(ps.tensor,
                                                    ps.offset + half * Wd,
                                                    [(psp, 128), (1, Wd)]),
                                        lhsT=bass.AP(ident.tensor, ident.offset,
                                                     [(ident.ap[0][0], 128), (1, 128)]),
                                        rhs=bass.AP(prod.tensor,
                                                    prod.offset + j * XP + half * Wd,
                                                    [(ppr, 128), (1, Wd)]),
                                        start=(bi == 0 and j == 0),
                                        stop=(bi == K - 1 and j == K - 1),
                                        skip_group_check=True)
                                    nmm += 1

                        # evac: deinterleave parity and multiply by W
                        #   acc[p, par*512+x] = ps[p, 2x+par] * W16[p, 2x+par]
                        nc.vector.tensor_mul(
                            out=bass.AP(acc.tensor, acc.offset,
                                        [(ap_, 128), (Wd, 2), (1, Wd)]),
                            in0=bass.AP(ps.tensor, ps.offset,
                                        [(psp, 128), (1, 2), (2, Wd)]),
                            in1=bass.AP(wt16, 2 * wt32.offset,
                                        [(2 * wtp, 128), (1, 2), (2, Wd)]))
                        # store: out[s, 2p+par, x] = acc[p, par*512+x]
                        nc.scalar.dma_start(
                            out=bass.AP(out_d.ap().tensor,
                                        s * H * Wd + p0 * XP,
                                        [(XP, 128), (Wd, 2), (1, Wd)]),
                            in_=bass.AP(acc.tensor, acc.offset,
                                        [(ap_, 128), (Wd, 2), (1, Wd)]))

    nc.finalize()
    return nc


def _make_runner():
    import jax
    from jax.sharding import Mesh, PartitionSpec, NamedSharding
    from jax.experimental.shard_map import shard_map
    from concourse import bass2jax

    bass2jax.install_neuronx_cc_hook()
    nc = _build_program()

    partition_name = (nc.partition_id_tensor.name
                      if nc.partition_id_tensor is not None else None)
    in_names, out_names, out_avals = [], [], []
    for alloc in nc.m.functions[0].allocations:
        if not isinstance(alloc, mybir.MemoryLocationSet):
            continue
        name = alloc.memorylocations[0].name
        if alloc.kind == "ExternalInput":
            if name != partition_name:
                in_names.append(name)
        elif alloc.kind == "ExternalOutput":
            out_names.append(name)
            out_avals.append(jax.core.ShapedArray(tuple(alloc.tensor_shape),
                                                  mybir.dt.np(alloc.dtype)))
    n_params = len(in_names)
    all_in_names = in_names + out_names
    if partition_name is not None:
        all_in_names = all_in_names + [partition_name]

    def _body(*args):
        operands = list(args)
        if partition_name is not None:
            operands.append(bass2jax.partition_id_tensor())
        outs = bass2jax._bass_exec_p.bind(
            *operands,
            out_avals=tuple(out_avals),
            in_names=tuple(all_in_names),
            out_names=tuple(out_names),
            lowering_input_output_aliases=(),
            sim_require_finite=True,
            sim_require_nnan=True,
            nc=nc,
        )
        return tuple(outs)

    devices = jax.devices()[:N_CORES]
    mesh = Mesh(np.asarray(devices), ("core",))
    spec = PartitionSpec("core")
    n_outs = len(out_names)
    fn = jax.jit(
        shard_map(_body, mesh=mesh, in_specs=(spec,) * (n_params + n_outs),
                  out_specs=(spec,) * n_outs, check_rep=False),
        keep_unused=True,
    )
    sharding = NamedSharding(mesh, spec)
    return fn, in_names, out_names, out_avals, sharding


_RUNNER = None


def _get_runner():
    global _RUNNER
    if _RUNNER is None:
        _RUNNER = _make_runner()
    return _RUNNER


_IDENT = None


def _pack_inputs(frames, core, w):
    """Full f32 inputs -> y-pair-packed f32-word arrays keyed by DRAM name."""
    global _IDENT
    frames = np.asarray(frames, dtype=np.float32)
    core = np.asarray(core, dtype=np.float32)
    w = np.asarray(w, dtype=np.float32)

    f16 = frames.reshape(B * N, H, Wd).astype(np.float16)
    fpad = np.zeros((B * N, HP, WP), dtype=np.float16)
    fpad[:, 2:2 + H, 2:2 + Wd] = f16

    # corey[m, q, p, x] = pack2(core16[m, q, 2p, x], core16[m, q, 2p+1, x])
    c16 = core.reshape(B * N, K2, H, Wd).astype(np.float16)
    corey = np.ascontiguousarray(
        c16.reshape(B * N, K2, HY, 2, Wd).transpose(0, 1, 2, 4, 3)
    ).view(np.float32).reshape(B * N, K2, HY, Wd)

    # frY[m, i, p, c] = pack2(fpad[m, 2p+i, c], fpad[m, 2p+1+i, c])
    sm, sr, sc = fpad.strides
    frv = np.lib.stride_tricks.as_strided(
        fpad, shape=(B * N, K, HY, WP, 2), strides=(sm, sr, 2 * sr, sc, sr))
    fry = np.ascontiguousarray(frv).view(np.float32).reshape(B * N, K, HY, WP)

    w16 = w.reshape(B * N, H, Wd).astype(np.float16)
    wy = np.ascontiguousarray(
        w16.reshape(B * N, HY, 2, Wd).transpose(0, 1, 3, 2)
    ).view(np.float32).reshape(B * N, HY, Wd)

    if _IDENT is None:
        _IDENT = np.tile(np.eye(128, dtype=np.float16), (N_CORES, 1, 1)).reshape(
            N_CORES * 128, 128)
    return {
        "corey": corey,
        "fry": fry,
        "wy": wy,
        "ident": _IDENT,
    }


_ZEROS = None


def _get_zeros(out_avals, sharding):
    global _ZEROS
    if _ZEROS is None:
        import jax
        _ZEROS = [jax.device_put(
            np.zeros((N_CORES * a.shape[0],) + tuple(a.shape[1:]), a.dtype),
            sharding) for a in out_avals]
    return _ZEROS


def kernel(**inputs):
    import jax

    fn, in_names, out_names, out_avals, sharding = _get_runner()
    packed = _pack_inputs(inputs["frames"], inputs["core"], inputs["W"])
    args = [jax.device_put(packed[name], sharding) for name in in_names]
    zeros = _get_zeros(out_avals, sharding)
    outs = fn(*args, *zeros)
    out = np.asarray(outs[out_names.index("out")])
    return out.reshape(B, N, H, Wd)


def benchmark(inputs, iters=10):
    """Best wall-clock ns per 8-core kernel execution (inputs resident on
    device, compile excluded)."""
    import jax, time

    fn, in_names, out_names, out_avals, sharding = _get_runner()
    packed = _pack_inputs(inputs["frames"], inputs["core"], inputs["W"])
    args = [jax.device_put(packed[name], sharding) for name in in_names]
    zeros = [jax.device_put(
        np.zeros((N_CORES * a.shape[0],) + tuple(a.shape[1:]), a.dtype), sharding)
        for a in out_avals]
    jax.block_until_ready(args)
    jax.block_until_ready(zeros)
    jax.block_until_ready(fn(*args, *zeros))
    best = float("inf")
    for _ in range(iters):
        t0 = time.perf_counter()
        jax.block_until_ready(fn(*args, *zeros))
        best = min(best, time.perf_counter() - t0)
    return int(best * 1e9)


# revision 3
# speedup vs baseline: 1.4275x; 1.4275x over previous
"""Trainium2 Bass kernel for nn_KernelConv (per-pixel dynamic 5x5 conv), v6.

  out[b,n,y,x] = W[b,n,y,x] * sum_{i,j} core[b, n*25+i*5+j, y, x] * frames_pad[b, n, y+i-2, x+j-2]

Sharding: pure data parallel; 16 (b,n) slices split 2-per-core across 8 cores.

v7 — y-pair packed fp16 (v6) + overlap tuning: all products on DVE, a
deeper 5-buffer pool for the core/product tiles, and the frame/W loads moved
to the Activation HWDGE queue so they never queue behind the core stream.
Measured 111.5us/core true device time (3.0x over the f32 baseline).

v6 — y-pair packed fp16:
  On this rig the DMA sustains ~300GB/s only for the planar geometry (2KB
  descriptors, strided source); packed/contiguous and 1KB-fp16 layouts cap
  near 65-85GB/s. So all inputs are fp16 values packed TWO ADJACENT Y-ROWS
  per float32 word, keeping the exact fast descriptor geometry at half the
  bytes (measured: core load 166us f32 -> 98us y-paired).

  One SBUF partition row p holds output rows {2p, 2p+1} interleaved as
  (x, parity) pairs; a strip covers 256 output rows, 2 strips per slice.
  - host packs: corey[m,q,p,x] = pack2(core[m,q,2p,x], core[m,q,2p+1,x]);
    frY[m,i,p,c] = pack2(fpad[m,2p+i,c], fpad[m,2p+1+i,c]) (the 5 kernel-row
    shifts are host-materialized since engine partition bases are
    quadrant-locked); wY likewise.
  - products (fp16, via bitcast views of the f32 tiles) on DVE (4 kernel
    rows) + Pool/gpsimd (1 row); the j-shift is a stride-2 overlapping AP.
  - 25-plane reduction on the otherwise-idle PE: identity matmuls
    accumulating into a 2-bank PSUM tile (one bank per 512-wide half).
  - evacuation deinterleaves parity while multiplying by W (DVE), and the
    output store runs on the Activation HWDGE queue (2KB descriptors).
"""

import numpy as np

import concourse.bacc as bacc
import concourse.bass as bass
import concourse.mybir as mybir
import concourse.tile as tile

F32 = mybir.dt.float32
F16 = mybir.dt.float16

B, N, H, Wd = 2, 8, 512, 512
K = 5
K2 = K * K
SLICES_PER_CORE = 2
HP = H + 4
WP = Wd + 4
N_CORES = 8
HY = H // 2                   # 256 y-pair rows per slice
XP = 2 * Wd                   # (x, parity) fp16 elements per pair-row
POOL_I = 0                    # kernel-row block computed on Pool; rest on DVE


def _build_program(reps=1):
    nc = bacc.Bacc("TRN2", target_bir_lowering=False)
    core_d = nc.dram_tensor("corey", (SLICES_PER_CORE, K2, HY, Wd), F32, kind="ExternalInput")
    fr_d = nc.dram_tensor("fry", (SLICES_PER_CORE, K, HY, WP), F32, kind="ExternalInput")
    w_d = nc.dram_tensor("wy", (SLICES_PER_CORE, HY, Wd), F32, kind="ExternalInput")
    id_d = nc.dram_tensor("ident", (128, 128), F16, kind="ExternalInput")
    out_d = nc.dram_tensor("out", (SLICES_PER_CORE, H, Wd), F32, kind="ExternalOutput")

    with tile.TileContext(nc) as tc:
        with tc.tile_pool(name="const", bufs=1) as cpool, \
             tc.tile_pool(name="big", bufs=5) as bpool, \
             tc.tile_pool(name="sml", bufs=4) as pool, \
             tc.psum_pool(name="ps", bufs=2) as pp:
            ident = cpool.tile([128, 128], F16, tag="ident")
            nc.sync.dma_start(out=ident, in_=id_d[:, :])

            for rep in range(reps):
                for s in range(SLICES_PER_CORE):
                    for p0 in (0, 128):
                        fw32 = pool.tile([128, K * WP], F32, tag="FW")
                        wt32 = pool.tile([128, Wd], F32, tag="Wt")
                        acc = pool.tile([128, XP], F32, tag="acc")
                        ps = pp.tile([128, XP], F32, tag="psum")

                        fwp = fw32.ap[0][0]
                        wtp = wt32.ap[0][0]
                        psp = ps.ap[0][0]
                        ap_ = acc.ap[0][0]
                        fw16 = fw32.tensor.bitcast(F16)
                        wt16 = wt32.tensor.bitcast(F16)

                        # frame windows (all 5 shifts): FW[p, i*WP+c] word
                        nc.scalar.dma_start(
                            out=bass.AP(fw32.tensor, fw32.offset,
                                        [(fwp, 128), (WP, K), (1, WP)]),
                            in_=fr_d[s, :, p0:p0 + 128, :].transpose([1, 0, 2]))
                        nc.scalar.dma_start(
                            out=bass.AP(wt32.tensor, wt32.offset, [(wtp, 128), (1, Wd)]),
                            in_=w_d[s, p0:p0 + 128, :])

                        blocks = []
                        for i in range(K):
                            ct32 = bpool.tile([128, K * Wd], F32, tag="C")
                            prod = bpool.tile([128, K * XP], F16, tag="prod")
                            cp = ct32.ap[0][0]
                            ppr = prod.ap[0][0]
                            ct16 = ct32.tensor.bitcast(F16)
                            # core i-block: C[p, j*512+x] word = corey[s,5i+j,p0+p,x]
                            nc.sync.dma_start(
                                out=bass.AP(ct32.tensor, ct32.offset,
                                            [(cp, 128), (Wd, K), (1, Wd)]),
                                in_=core_d[s, i * K:(i + 1) * K,
                                           p0:p0 + 128, :].transpose([1, 0, 2]))
                            # product: prod[p, j*XP + 2x+par] =
                            #   C[p, (j*512+x)*2+par] * FW[p, (i*WP + x+j)*2+par]
                            nc.vector.tensor_mul(
                                out=bass.AP(prod.tensor, prod.offset,
                                            [(ppr, 128), (XP, K), (1, XP)]),
                                in0=bass.AP(ct16, 2 * ct32.offset,
                                            [(2 * cp, 128), (XP, K), (1, XP)]),
                                in1=bass.AP(fw16, 2 * (fw32.offset + i * WP),
                                            [(2 * fwp, 128), (2, K), (1, XP)]))
                            blocks.append((i, prod, ppr))

                        # PE: accumulate the 25 product planes into 2 PSUM
                        # bank halves (512 f32 per partition each)
                        nmm = 0
                        for bi, (i, prod, ppr) in enumerate(blocks):
                            for j in range(K):
                                for half in range(2):
                                    nc.tensor.matmul(
                                        out=bass.AP(ps.tensor,
                                                    ps.offset + half * Wd,
                                                    [(psp, 128), (1, Wd)]),
                                        lhsT=bass.AP(ident.tensor, ident.offset,
                                                     [(ident.ap[0][0], 128), (1, 128)]),
                                        rhs=bass.AP(prod.tensor,
                                                    prod.offset + j * XP + half * Wd,
                                                    [(ppr, 128), (1, Wd)]),
                                        start=(bi == 0 and j == 0),
                                        stop=(bi == K - 1 and j == K - 1),
                                        skip_group_check=True)
                                    nmm += 1

                        # evac: deinterleave parity and multiply by W
                        #   acc[p, par*512+x] = ps[p, 2x+par] * W16[p, 2x+par]
                        nc.vector.tensor_mul(
                            out=bass.AP(acc.tensor, acc.offset,
                                        [(ap_, 128), (Wd, 2), (1, Wd)]),
                            in0=bass.AP(ps.tensor, ps.offset,
                                        [(psp, 128), (1, 2), (2, Wd)]),
                            in1=bass.AP(wt16, 2 * wt32.offset,
                                        [(2 * wtp, 128), (1, 2), (2, Wd)]))
                        # store: out[s, 2p+par, x] = acc[p, par*512+x]
                        nc.scalar.dma_start(
                            out=bass.AP(out_d.ap().tensor,
                                        s * H * Wd + p0 * XP,
                                        [(XP, 128), (Wd, 2), (1, Wd)]),
                            in_=bass.AP(acc.tensor, acc.offset,
                                        [(ap_, 128), (Wd, 2), (1, Wd)]))

    nc.finalize()
    return nc


def _make_runner():
    import jax
    from jax.sharding import Mesh, PartitionSpec, NamedSharding
    from jax.experimental.shard_map import shard_map
    from concourse import bass2jax

    bass2jax.install_neuronx_cc_hook()
    nc = _build_program()

    partition_name = (nc.partition_id_tensor.name
                      if nc.partition_id_tensor is not None else None)
    in_names, out_names, out_avals = [], [], []
    for alloc in nc.m.functions[0].allocations:
        if not isinstance(alloc, mybir.MemoryLocationSet):
            continue
        name = alloc.memorylocations[0].name
        if alloc.kind == "ExternalInput":
            if name != partition_name:
                in_names.append(name)
        elif alloc.kind == "ExternalOutput":
            out_names.append(name)
            out_avals.append(jax.core.ShapedArray(tuple(alloc.tensor_shape),
                                                  mybir.dt.np(alloc.dtype)))
    n_params = len(in_names)
    all_in_names = in_names + out_names
    if partition_name is not None:
        all_in_names = all_in_names + [partition_name]

    def _body(*args):
        operands = list(args)
        if partition_name is not None:
            operands.append(bass2jax.partition_id_tensor())
        outs = bass2jax._bass_exec_p.bind(
            *operands,
            out_avals=tuple(out_avals),
            in_names=tuple(all_in_names),
            out_names=tuple(out_names),
            lowering_input_output_aliases=(),
            sim_require_finite=True,
            sim_require_nnan=True,
            nc=nc,
        )
        return tuple(outs)

    devices = jax.devices()[:N_CORES]
    mesh = Mesh(np.asarray(devices), ("core",))
    spec = PartitionSpec("core")
    n_outs = len(out_names)
    fn = jax.jit(
        shard_map(_body, mesh=mesh, in_specs=(spec,) * (n_params + n_outs),
                  out_specs=(spec,) * n_outs, check_rep=False),
        keep_unused=True,
    )
    sharding = NamedSharding(mesh, spec)
    return fn, in_names, out_names, out_avals, sharding


_RUNNER = None


def _get_runner():
    global _RUNNER
    if _RUNNER is None:
        _RUNNER = _make_runner()
    return _RUNNER


_IDENT = None


def _pack_inputs(frames, core, w):
    """Full f32 inputs -> y-pair-packed f32-word arrays keyed by DRAM name."""
    global _IDENT
    frames = np.asarray(frames, dtype=np.float32)
    core = np.asarray(core, dtype=np.float32)
    w = np.asarray(w, dtype=np.float32)

    f16 = frames.reshape(B * N, H, Wd).astype(np.float16)
    fpad = np.zeros((B * N, HP, WP), dtype=np.float16)
    fpad[:, 2:2 + H, 2:2 + Wd] = f16

    # corey[m, q, p, x] = pack2(core16[m, q, 2p, x], core16[m, q, 2p+1, x])
    c16 = core.reshape(B * N, K2, H, Wd).astype(np.float16)
    corey = np.ascontiguousarray(
        c16.reshape(B * N, K2, HY, 2, Wd).transpose(0, 1, 2, 4, 3)
    ).view(np.float32).reshape(B * N, K2, HY, Wd)

    # frY[m, i, p, c] = pack2(fpad[m, 2p+i, c], fpad[m, 2p+1+i, c])
    sm, sr, sc = fpad.strides
    frv = np.lib.stride_tricks.as_strided(
        fpad, shape=(B * N, K, HY, WP, 2), strides=(sm, sr, 2 * sr, sc, sr))
    fry = np.ascontiguousarray(frv).view(np.float32).reshape(B * N, K, HY, WP)

    w16 = w.reshape(B * N, H, Wd).astype(np.float16)
    wy = np.ascontiguousarray(
        w16.reshape(B * N, HY, 2, Wd).transpose(0, 1, 3, 2)
    ).view(np.float32).reshape(B * N, HY, Wd)

    if _IDENT is None:
        _IDENT = np.tile(np.eye(128, dtype=np.float16), (N_CORES, 1, 1)).reshape(
            N_CORES * 128, 128)
    return {
        "corey": corey,
        "fry": fry,
        "wy": wy,
        "ident": _IDENT,
    }


_ZEROS = None


def _get_zeros(out_avals, sharding):
    global _ZEROS
    if _ZEROS is None:
        import jax
        _ZEROS = [jax.device_put(
            np.zeros((N_CORES * a.shape[0],) + tuple(a.shape[1:]), a.dtype),
            sharding) for a in out_avals]
    return _ZEROS


def kernel(**inputs):
    import jax

    fn, in_names, out_names, out_avals, sharding = _get_runner()
    packed = _pack_inputs(inputs["frames"], inputs["core"], inputs["W"])
    args = [jax.device_put(packed[name], sharding) for name in in_names]
    zeros = _get_zeros(out_avals, sharding)
    outs = fn(*args, *zeros)
    out = np.asarray(outs[out_names.index("out")])
    return out.reshape(B, N, H, Wd)


def benchmark(inputs, iters=10):
    """Best wall-clock ns per 8-core kernel execution (inputs resident on
    device, compile excluded)."""
    import jax, time

    fn, in_names, out_names, out_avals, sharding = _get_runner()
    packed = _pack_inputs(inputs["frames"], inputs["core"], inputs["W"])
    args = [jax.device_put(packed[name], sharding) for name in in_names]
    zeros = [jax.device_put(
        np.zeros((N_CORES * a.shape[0],) + tuple(a.shape[1:]), a.dtype), sharding)
        for a in out_avals]
    jax.block_until_ready(args)
    jax.block_until_ready(zeros)
    jax.block_until_ready(fn(*args, *zeros))
    best = float("inf")
    for _ in range(iters):
        t0 = time.perf_counter()
        jax.block_until_ready(fn(*args, *zeros))
        best = min(best, time.perf_counter() - t0)
    return int(best * 1e9)
